# revision 2
# baseline (speedup 1.0000x reference)
"""Trainium2 Bass kernel for Enformer-style relative-position attention.

Problem: nn_Attention_79087527788690
  x [1, 2048, 1536] -> out [1, 2048, 1536]
  8 heads, dk=64, dv=192, rel-pos features=192, n=2048.

Sharding: one head per NeuronCore (8 cores). Each core computes its head's
q/k/v projections, content + relative-position logits, softmax weights and
per-head attention output oh [2048, 192]. Transposed oh row-tiles are
exchanged via two AllToAll collectives (tiles 0-7, then 8-15; core c owns
tiles {c, 8+c}); each core then multiplies its two owned row-tiles against
the full (dv-swizzled) Wo for its final [256, 1536] output rows.

Transposed-native attention pipeline: content logits are computed already
transposed (kT stationary per j-tile, qcT moving) so exp lands as E0^T
[j-part, i-free]; the relative-position shear round trip reads back through
the DMA xbar TRANSPOSE (out[p,b,i] = shifted[i, 128b+p]) so exp(R)^T
arrives in the same [128, 16, 128] block layout; E^T = E0^T * relT on DVE.
PV then consumes E^T blocks directly as matmul stationaries - the 256
PE transposes + PSUM evacuations of the row-major variant disappear.

relk (= positional_embed @ Wrel, a weight-only transformation independent
of x) is precomputed host-side per head and fed replicated, per the
sharding hint.
"""

import math
import os
import sys
from contextlib import ExitStack

sys.path.insert(0, "/opt/trn_rl_repo")

import numpy as np

N = 2048
DIM = 1536
HEADS = 8
DK = 64
DV = 192
F = 192  # rel pos features
SPAN = 2 * N - 1  # 4095
NCORES = 8
CHUNK = N // NCORES  # 256
SCALE = DK ** -0.5

K_TRAIL = int(os.environ.get("K_TRAIL", "1"))
K_LEAD = int(os.environ.get("K_LEAD", "4"))
K_EB = int(os.environ.get("K_EB", "4"))
K_GB = int(os.environ.get("K_GB", "5"))
K_RB = int(os.environ.get("K_RB", "5"))
K_FB = int(os.environ.get("K_FB", "2"))
K_WOFIT = int(os.environ.get("K_WOFIT", "3"))
K_PPB = int(os.environ.get("K_PPB", "1"))  # outproj psum ring depth
K_PCB = int(os.environ.get("K_PCB", "2"))  # content psum ring depth
K_POB = int(os.environ.get("K_POB", "1"))
K_FINE = os.environ.get("K_FINE", "pool")  # outproj psum evacuation engine
K_QCE = os.environ.get("K_QCE", "act")     # qc/qp bias evacuation engine
K_OP0 = os.environ.get("K_OP0", "drain")
K_OP0RING = os.environ.get("K_OP0RING", "pp")
K_COMM0 = int(os.environ.get("K_COMM0", "9"))
K_OUT16 = os.environ.get("K_OUT16", "1") == "1"
K_OPALT = os.environ.get("K_OPALT", "1") == "1"

IT = 128          # q rows per tile
NIT = N // IT     # 16
JC = 512          # j chunk for logits
NJC = N // JC     # 4
GW = N + IT - 1   # 2175, G window per i-tile
GPITCH = 2176     # padded pitch of the DRAM G buffer
NAG = 2           # all-to-all groups (tiles 0-7, 8-15)


def _positions() -> np.ndarray:
    """get_positional_embed(2048, 192) in numpy (f64 -> f32). [4095, 192]"""
    d = np.arange(-N + 1, N).astype(np.float64)
    nb = F // 6
    absd = np.abs(d)[:, None]
    max_range = math.log(N) / math.log(2.0)
    half_life = 2.0 ** np.linspace(3.0, max_range, nb)
    feat_exp = np.exp(-math.log(2.0) / half_life[None, :] * absd)
    cw = 2.0 ** np.arange(1, nb + 1) - 1.0
    feat_cm = (cw[None, :] > absd).astype(np.float64)
    stddev = N / (2 * nb)
    start_mean = N / nb
    mean = np.linspace(start_mean, N, nb)[None, :]
    conc = (mean / stddev) ** 2
    rate = mean / stddev ** 2
    with np.errstate(divide="ignore", invalid="ignore"):
        log_unnorm = (conc - 1.0) * np.log(absd) - rate * absd
    log_unnorm = np.where(absd == 0, -np.inf, log_unnorm)
    lg = np.vectorize(math.lgamma)(conc)
    log_norm = lg - conc * np.log(rate)
    probs = np.exp(log_unnorm - log_norm) + 1e-8
    feat_gamma = probs / np.amax(probs, axis=-1, keepdims=True)
    emb = np.concatenate([feat_exp, feat_cm, feat_gamma], axis=-1)
    out = np.concatenate([emb, np.sign(d)[:, None] * emb], axis=-1)
    return out.astype(np.float32)


def build_nc(num_cores: int = NCORES, collective: bool = True):
    """Build + compile the per-core Bass graph (SPMD, identical on all cores)."""
    import concourse.bass as bass
    import concourse.mybir as mybir
    import concourse.tile as tile
    from concourse import bacc
    from concourse.masks import make_identity

    f32 = mybir.dt.float32
    f16 = mybir.dt.float16
    bf16 = mybir.dt.bfloat16
    ODT = f16 if K_OUT16 else f32

    nc = bacc.Bacc(
        "TRN2", target_bir_lowering=False, debug=False, num_devices=num_cores
    )

    # --- external I/O (per-core shards supplied via in_maps) ---
    xT_e = nc.dram_tensor("xT", [DIM, N], f16, kind="ExternalInput")
    # wqk: [p, c, 0:64] = wq[128c+p] * SCALE ; [p, c, 64:128] = wk[128c+p]
    wqk_e = nc.dram_tensor("wqk", [128, 12, 2 * DK], f16, kind="ExternalInput")
    # wvp: [p, c, :] = wv[128c+p]
    wvp_e = nc.dram_tensor("wvp", [128, 12, DV], f16, kind="ExternalInput")
    # host-precomputed rel_k^T for this head: [64, 4095]
    relkT_e = nc.dram_tensor("relkT", [DK, SPAN], f16, kind="ExternalInput")
    rb2_e = nc.dram_tensor("rb2", [DK, 2], f32, kind="ExternalInput")
    # wof: full out-projection weight, dv-swizzled: [p, kc, :] = woP[128kc + p]
    wof_e = nc.dram_tensor("wof", [128, 12, DIM], f16, kind="ExternalInput")
    bo_e = nc.dram_tensor("bo", [1, DIM], f16, kind="ExternalInput")
    out_e = nc.dram_tensor("out", [CHUNK, DIM], ODT, kind="ExternalOutput")

    # --- internal DRAM ---
    # a2a slot o of group ag carries this core's ohT for tile 8*ag + o
    a2a_ins = [nc.dram_tensor(f"a2a_in{g}", [NCORES, DV, IT], f16) for g in range(NAG)]
    a2a_outs = [nc.dram_tensor(f"a2a_out{g}", [NCORES, DV, IT], f16) for g in range(NAG)]
    NGD = 5
    gds = [nc.dram_tensor(f"gd{i}", [IT, GPITCH], bf16) for i in range(NGD)]

    with tile.TileContext(nc) as tc, ExitStack() as ctx:
        const = ctx.enter_context(tc.tile_pool(name="const", bufs=1))
        work = ctx.enter_context(tc.tile_pool(name="work", bufs=2))
        psum = ctx.enter_context(tc.tile_pool(name="psum", bufs=2, space="PSUM"))

        # ---- constants / weights into SBUF ----
        ident_h = const.tile([128, 128], f16, tag="idh")
        make_identity(nc, ident_h[:])
        ones_r = const.tile([1, 128], f16, tag="onesr")
        nc.vector.memset(ones_r[:], 1.0)

        # x slices live in a ring: consumed once by the projections, then
        # the space recycles
        xs = [work.tile([128, 12, 512], f16, tag="xs", bufs=3, name=f"xs{sl}")
              for sl in range(4)]
        wqk_s = const.tile([128, 12, 2 * DK], f16, tag="wqk")
        wv_s = const.tile([128, 12, DV], f16, tag="wv")
        relkT = const.tile([DK, SPAN + 1], f16, tag="relkT")
        rb2_s = const.tile([DK, 2], f32, tag="rb2")
        bo_r = const.tile([1, DIM], f16, tag="bor")
        # Arrival-matched load order: wqk + relkT + x slice 0 first
        # (c-granular so the first q/k projection starts consuming chunks as
        # they land), then wv and the remaining x slices; bulky non-urgent
        # constants (wof, bo) deferred to mid-loop.
        nc.sync.dma_start(out=wqk_s[:], in_=wqk_e[:])
        nc.sync.dma_start(out=rb2_s[:], in_=rb2_e[:])
        nc.sync.dma_start(out=relkT[:, 0:SPAN], in_=relkT_e[:])
        for c in range(12):
            nc.sync.dma_start(out=xs[0][:, c, :], in_=xT_e[128 * c:128 * (c + 1), 0:512])
        nc.sync.dma_start(out=wv_s[:], in_=wvp_e[:])
        for sl in range(1, 4):
            nc.sync.dma_start(
                out=xs[sl][:],
                in_=bass.AP(xT_e, 512 * sl,
                            [[N, 128], [N * 128, 12], [1, 512]]))
        wof_p = [work.tile([128, 4, DIM], f16, tag="xs", bufs=3, name=f"wof{j}")
                 for j in range(3)]

        def emit_wof_load():
            nc.sync.dma_start(out=bo_r[:], in_=bo_e[:])
            for j in range(3):
                nc.sync.dma_start(out=wof_p[j][:],
                                  in_=wof_e[:, 4 * j:4 * (j + 1), :])

        # ---- projections ----
        qcT = const.tile([DK, N], f16, tag="qcT")  # (q*s + rcb)^T
        qpT = const.tile([DK, N], f16, tag="qpT")  # (q*s + rpb)^T
        kT = const.tile([DK, N], f16, tag="kT")
        # j-tile jt at [:, jt, :]; column DV is all-ones so the PV matmul
        # also accumulates the softmax row sums (po[:, DV])
        vb = const.tile([128, NIT, DV + 1], bf16, tag="vb")
        nc.vector.memset(vb[:, :, DV:DV + 1], 1.0)

        def emit_qk(ic):
            pq = psum.tile([128, 512], f32, tag="pc", name=f"pq{ic}")
            for c in range(12):
                nc.tensor.matmul(pq[:], wqk_s[:, c, :],
                                 xs[ic][:, c, :],
                                 start=(c == 0), stop=(c == 11))
            if K_QCE == "dve":
                nc.vector.tensor_scalar(qcT[:, 512 * ic:512 * (ic + 1)],
                                        pq[0:DK, :], rb2_s[:, 0:1], None,
                                        mybir.AluOpType.add)
                nc.vector.tensor_scalar(qpT[:, 512 * ic:512 * (ic + 1)],
                                        pq[0:DK, :], rb2_s[:, 1:2], None,
                                        mybir.AluOpType.add)
            else:
                nc.scalar.activation(qcT[:, 512 * ic:512 * (ic + 1)], pq[0:DK, :],
                                     mybir.ActivationFunctionType.Identity,
                                     bias=rb2_s[:, 0:1], scale=1.0)
                nc.scalar.activation(qpT[:, 512 * ic:512 * (ic + 1)], pq[0:DK, :],
                                     mybir.ActivationFunctionType.Identity,
                                     bias=rb2_s[:, 1:2], scale=1.0)
            nc.vector.tensor_copy(kT[:, 512 * ic:512 * (ic + 1)], pq[DK:2 * DK, :])

        def emit_v(jt):
            pv = psum.tile([128, DV], f32, tag="po", bufs=K_POB, name=f"pv{jt}")
            for c in range(12):
                nc.tensor.matmul(pv[:], xs[jt // 4][:, c, IT * (jt % 4):IT * (jt % 4 + 1)],
                                 wv_s[:, c, :], start=(c == 0), stop=(c == 11))
            nc.vector.tensor_copy(vb[:, jt, 0:DV], pv[:])

        # ---- G stage: rel-logit window matmuls + exp + sheared-transposed
        # DRAM round trip. relT[p, b, i] = exp(R)[i0+i, 128b+p]. ----
        def emit_g(it):
            i0 = IT * it
            w0 = (N - IT) - i0  # window start s0 = 1920 - i0
            gwin = work.tile([128, GPITCH], bf16, tag="gwin", bufs=K_GB, name=f"gwin{it}")
            for q in range(4):
                pg = psum.tile([128, JC], f32, tag="pg", name=f"pg{it}_{q}")
                nc.tensor.matmul(pg[:], qpT[:, i0:i0 + IT],
                                 relkT[:, w0 + JC * q:w0 + JC * (q + 1)],
                                 start=True, stop=True)
                nc.scalar.activation(gwin[:, JC * q:JC * (q + 1)], pg[:],
                                     mybir.ActivationFunctionType.Exp)
            pg2 = psum.tile([128, IT], f32, tag="pp", bufs=K_PPB, name=f"pg2_{it}")
            nc.tensor.matmul(pg2[:, 0:IT - 1], qpT[:, i0:i0 + IT],
                             relkT[:, w0 + 4 * JC:w0 + GW], start=True, stop=True)
            nc.scalar.activation(gwin[:, 4 * JC:GW], pg2[:, 0:IT - 1],
                                 mybir.ActivationFunctionType.Exp)
            gd = gds[it % NGD]
            nc.sync.dma_start(out=gd[:, 0:GW], in_=gwin[:, 0:GW])
            diag = bass.AP(gd, 127, [[GW, 128], [1, N]])
            relT = work.tile([128, NIT, IT], bf16, tag="relT", bufs=K_RB,
                             name=f"relT{it}")
            nc.sync.dma_start_transpose(out=relT[:], in_=diag)
            return relT

        # ---- content logits (transposed) + exp + E^T = expC^T * expR^T ----
        def emit_logits(it, relT):
            i0 = IT * it
            ET = work.tile([128, NIT, IT], bf16, tag="E", bufs=K_EB, name=f"E{it}")
            for jq in range(NJC):
                pcT = psum.tile([128, 4, IT], f32, tag="pc", bufs=K_PCB,
                                name=f"pcT{it}_{jq}")
                for q in range(4):
                    jt = 4 * jq + q
                    nc.tensor.matmul(pcT[:, q, :], kT[:, IT * jt:IT * (jt + 1)],
                                     qcT[:, i0:i0 + IT], start=True, stop=True)
                E0T = work.tile([128, 4, IT], bf16, tag="E0", bufs=3,
                                name=f"E0_{it}_{jq}")
                nc.scalar.activation(E0T[:], pcT[:],
                                     mybir.ActivationFunctionType.Exp)
                nc.vector.tensor_tensor(ET[:, 4 * jq:4 * (jq + 1), :], E0T[:],
                                        relT[:, 4 * jq:4 * (jq + 1), :],
                                        mybir.AluOpType.mult)
            return (ET,)

        # ---- PV + rowsum column + ohT all-to-all send for tile it ----
        def emit_pv(it, ET):
            po = psum.tile([128, DV + 1], f32, tag="po", bufs=K_POB, name=f"po{it}")
            for jt in range(NIT):
                nc.tensor.matmul(po[:], ET[:, jt, :], vb[:, jt, :],
                                 start=(jt == 0), stop=(jt == NIT - 1))
            rcp = work.tile([128, 1], f32, tag="rcp", bufs=2, name=f"rcp{it}")
            nc.vector.reciprocal(rcp[:], po[:, DV:DV + 1])
            oh = work.tile([128, DV], f16, tag="oh", name=f"oh{it}")
            nc.vector.tensor_scalar(oh[:], po[:, 0:DV], rcp[:], None,
                                    mybir.AluOpType.mult)
            # transpose oh -> ohT (c-chunks of 96)
            ohT = work.tile([96, 2, 128], f16, tag="ohT", name=f"ohT{it}")
            for h in range(2):
                pth = psum.tile([96, 128], f16, tag="tr4", bufs=2,
                                name=f"pth{it}_{h}")
                nc.tensor.transpose(pth[:], oh[:, 96 * h:96 * (h + 1)], ident_h[:])
                nc.vector.tensor_copy(ohT[:, h, :], pth[:])
            # send this tile's ohT into its all-to-all staging slot
            ag = it // 8
            nc.sync.dma_start(
                out=bass.AP(a2a_ins[ag], (it % 8) * DV * IT,
                            [[IT, 96], [96 * IT, 2], [1, IT]]),
                in_=ohT[:])

        def emit_a2a_comm(ag):
            # Exchange ohT tiles: slot o -> core o (owner of tile 8*ag+o),
            # then gather the 8 heads' [192, 128] blocks into the
            # [128, 12, 128] dv-swizzled stationary layout: chunk h (0-7) =
            # head h dv 0:128; chunk 8+k = heads (2k, 2k+1) dv 128:192.
            if collective:
                nc.gpsimd.collective_compute(
                    "AllToAll",
                    mybir.AluOpType.bypass,
                    replica_groups=[list(range(num_cores))],
                    ins=[a2a_ins[ag][:]],
                    outs=[a2a_outs[ag][:]],
                )
                cc_src = a2a_outs[ag]
            else:
                cc_src = a2a_ins[ag]  # timing mirror: same local read traffic
            agb = work.tile([128, 12, IT], f16, tag="agb", bufs=2, name=f"agb{ag}")
            HB = DV * IT  # one head slot
            nc.sync.dma_start(
                out=agb[:, 0:8, :],
                in_=bass.AP(cc_src, 0, [[IT, 128], [HB, 8], [1, IT]]))
            for b in range(2):
                nc.sync.dma_start(
                    out=agb[64 * b:64 * (b + 1), 8:12, :],
                    in_=bass.AP(cc_src, b * HB + 128 * IT,
                                [[IT, 64], [2 * HB, 4], [1, IT]]))
            return agb

        def emit_outproj(ag, agb, tag, bufs):
            # Final out rows for owned tile 8*ag + core_id: agb @ woP + bo.
            fin = work.tile([128, 3, JC], ODT, tag="fin", bufs=K_FB, name=f"fin{ag}")
            rings = [(tag, bufs), ("po", K_POB), (tag, bufs)] if K_OPALT \
                else [(tag, bufs)] * 3
            for cc in range(3):
                rt, rb = rings[cc]
                pp = psum.tile([128, JC], f32, tag=rt, bufs=rb,
                               name=f"ppo{ag}_{cc}")
                # bias start-pass: ones-column x bo row seeds PSUM with the
                # broadcast bias, so evacuation is a plain copy
                nc.tensor.matmul(pp[:], ones_r[:, 0:128],
                                 bo_r[:, JC * cc:JC * (cc + 1)],
                                 start=True, stop=False)
                for kc in range(12):
                    nc.tensor.matmul(pp[:], agb[:, kc, :],
                                     wof_p[kc // 4][:, kc % 4, JC * cc:JC * (cc + 1)],
                                     start=False, stop=(kc == 11))
                if K_FINE == "act":
                    nc.scalar.copy(fin[:, cc, :], pp[:])
                elif K_FINE == "pool":
                    nc.gpsimd.tensor_copy(fin[:, cc, :], pp[:])
                else:
                    nc.vector.tensor_copy(fin[:, cc, :], pp[:])
                nc.sync.dma_start(out=out_e[IT * ag:IT * (ag + 1),
                                            JC * cc:JC * (cc + 1)],
                                  in_=fin[:, cc, :])

        # ---- drive: G leads by K_LEAD tiles, PV trails logits by K_TRAIL ----
        emit_qk(0)
        rel_q = [emit_g(i) for i in range(K_LEAD)]
        for jt in range(4):
            emit_v(jt)
        for ic in range(1, 4):
            emit_qk(ic)
            for jt in range(4 * ic, 4 * ic + 4):
                emit_v(jt)

        pv_q = []
        agb0 = None
        for it in range(NIT):
            relT = rel_q.pop(0)
            if len(pv_q) >= K_TRAIL:
                itp, eo = pv_q.pop(0)
                emit_pv(itp, *eo)
            if it == K_WOFIT:
                emit_wof_load()
            if it == K_COMM0:
                agb0 = emit_a2a_comm(0)
            pv_q.append((it, emit_logits(it, relT)))
            if it == 10 and K_OP0 == "steady":
                emit_outproj(0, agb0, K_OP0RING, K_PCB if K_OP0RING == "pc" else K_PPB)
            if it + K_LEAD < NIT:
                rel_q.append(emit_g(it + K_LEAD))
        for itp, eo in pv_q:
            emit_pv(itp, *eo)
        # group-0 outproj in the drain: its matmuls overlap the group-1
        # exchange -> gather DMA chain
        agb1 = emit_a2a_comm(1)
        if K_OP0 == "drain":
            emit_outproj(0, agb0, K_OP0RING, K_PCB if K_OP0RING == "pc" else K_PPB)
        emit_outproj(1, agb1, "pg", 2)

    nc.compile()
    return nc


_CACHE: dict = {}


def _get_nc():
    if "nc" not in _CACHE:
        _CACHE["nc"] = build_nc()
    return _CACHE["nc"]


def _shard_inputs(x, Wq, Wk, Wv, Wrel, rel_content_bias, rel_pos_bias, Wo, bo):
    positions = _positions()  # [4095, 192] f32
    relk_all = positions @ np.asarray(Wrel, np.float32)  # [4095, 8*64]
    xT = np.ascontiguousarray(
        np.asarray(x, np.float32).reshape(N, DIM).T).astype(np.float16)
    # dv-swizzled full Wo: chunk h (0-7) = head h dv 0:128; chunk 8+k =
    # heads (2k, 2k+1) dv 128:192 stacked 64+64 (matches emit_a2a_comm).
    woP = np.empty((DIM, DIM), np.float32)
    for h in range(8):
        woP[128 * h:128 * (h + 1)] = Wo[DV * h:DV * h + 128]
    for k in range(4):
        woP[1024 + 128 * k:1024 + 128 * k + 64] = Wo[DV * 2 * k + 128:DV * 2 * k + DV]
        woP[1024 + 128 * k + 64:1024 + 128 * (k + 1)] = \
            Wo[DV * (2 * k + 1) + 128:DV * (2 * k + 1) + DV]
    wof = np.ascontiguousarray(
        woP.reshape(12, 128, DIM).transpose(1, 0, 2)).astype(np.float16)
    bo_row = np.asarray(bo, np.float16).reshape(1, DIM)
    in_maps = []
    for h in range(NCORES):
        wq = (Wq[:, DK * h:DK * (h + 1)] * SCALE).astype(np.float16)
        wk = Wk[:, DK * h:DK * (h + 1)].astype(np.float16)
        wqk = np.concatenate(
            [wq.reshape(12, 128, DK), wk.reshape(12, 128, DK)], axis=2)
        wvp = Wv[:, DV * h:DV * (h + 1)].astype(np.float16).reshape(12, 128, DV)
        relkT_h = np.ascontiguousarray(
            relk_all[:, DK * h:DK * (h + 1)].T).astype(np.float16)
        rb2 = np.stack([rel_content_bias[0, h, 0, :],
                        rel_pos_bias[0, h, 0, :]], axis=1).astype(np.float32)
        in_maps.append({
            "xT": xT,
            "wqk": np.ascontiguousarray(wqk.transpose(1, 0, 2)),
            "wvp": np.ascontiguousarray(wvp.transpose(1, 0, 2)),
            "relkT": relkT_h,
            "rb2": np.ascontiguousarray(rb2),
            "wof": wof,
            "bo": bo_row,
        })
    return in_maps


def kernel(**inputs) -> np.ndarray:
    from concourse.bass_utils import run_bass_kernel_spmd

    inputs = {k: np.asarray(v) for k, v in inputs.items()}
    nc = _get_nc()
    in_maps = _shard_inputs(**inputs)
    res = run_bass_kernel_spmd(nc, in_maps, list(range(NCORES)))
    # core c owns row-tiles {c, 8+c}: rows [128c, 128c+128) and
    # [1024+128c, 1024+128c+128)
    out = np.empty((N, DIM), np.float32)
    for c in range(NCORES):
        oc = np.asarray(res.results[c]["out"]).astype(np.float32)
        out[IT * c:IT * (c + 1), :] = oc[0:IT, :]
        out[1024 + IT * c:1024 + IT * (c + 1), :] = oc[IT:2 * IT, :]
    return out.reshape(1, N, DIM)
